# revision 15
# baseline (speedup 1.0000x reference)
"""Causal self-attention with RoPE on 8 Trainium2 NeuronCores.

Sharding: tensor-parallel over heads (4 heads/core) x data-parallel over
batch (2 batches), 8 cores total.  Each core computes QKV projections for
its 4 heads from x[b].T, applies RoPE, runs causal attention, and produces
a partial output projection (row-parallel Wo); the host sums the 4 partials
per batch.

Per-core dataflow (all matmuls bf16 with fp32 PSUM accumulation):
  phase A: qT/kT = Wq_g @ xT (head-dim on partitions, real/imag
           de-interleaved via host-side weight row permutation so RoPE
           becomes partition-half operations), v = x @ Wv_g.T (natural)
  phase B: per head: RoPE rotate qT/kT; per 512-query block compute
           scores TRANSPOSED (k-major: lhsT=kT chunk, rhs=qT block) so
           softmax probabilities come out in the layout the PV matmul
           needs -- no PE transposes.  exp without max-subtraction
           (scores are bounded ~N(0,1)), causal mask on diagonal chunks,
           row-sums via a ones-vector matmul, normalization folded into
           the attnT copy-out as a column-broadcast multiply.
  phase C: partial out = attnT.T @ (Wo.T rows for this group) -> DRAM
"""

import sys

sys.path.insert(0, "/opt/trn_rl_repo")

import numpy as np
import ml_dtypes

import concourse.bass as bass
import concourse.mybir as mybir
import concourse.tile as tile
from concourse import bacc
from concourse.bass_utils import run_bass_kernel_spmd

B, C, D, H = 2, 2048, 2048, 16
HD = D // H            # 128 head dim
NCORE = 8
HPC = 4                # heads per core
GW = HPC * HD          # 512: per-core projection width
NKC = D // 128         # 16 contraction chunks
NMT = C // 128         # 16 query m-tiles
NBLK = C // 512        # 4 query blocks
SCALE = 1.0 / np.sqrt(HD)

bf16 = ml_dtypes.bfloat16
BF = mybir.dt.bfloat16
F32 = mybir.dt.float32

TRACE = False
TMPDIR = None
LAST = {}

_nc_cache = []


def _build_nc():
    nc = bacc.Bacc()

    xt_d = nc.declare_dram_parameter("xt", [D, C], BF, isOutput=False)
    wq_d = nc.declare_dram_parameter("wq", [D, GW], BF, isOutput=False)
    wk_d = nc.declare_dram_parameter("wk", [D, GW], BF, isOutput=False)
    wv_d = nc.declare_dram_parameter("wv", [D, GW], BF, isOutput=False)
    wo_d = nc.declare_dram_parameter("wo", [GW, D], BF, isOutput=False)
    cs_d = nc.declare_dram_parameter("cs", [128, C], BF, isOutput=False)
    sn_d = nc.declare_dram_parameter("sn", [128, C], BF, isOutput=False)
    mskT_d = nc.declare_dram_parameter("mskT", [128, 4 * 512], BF,
                                       isOutput=False)
    ones_d = nc.declare_dram_parameter("ones", [128, 1], BF, isOutput=False)
    out_d = nc.declare_dram_parameter("out", [C, D], F32, isOutput=True)

    with tile.TileContext(nc) as tc:
        with tc.tile_pool(name="consts", bufs=1) as cpool, \
             tc.tile_pool(name="vpool", bufs=1) as vpool, \
             tc.tile_pool(name="qkraw", bufs=1) as qkpool, \
             tc.tile_pool(name="rtmp", bufs=6) as rtmp:

            cs_t = cpool.tile([128, C], BF, name="cs_t")
            sn_t = cpool.tile([128, C], BF, name="sn_t")
            mskT_t = cpool.tile([128, 4 * 512], BF, name="mskT_t")
            ones_t = cpool.tile([128, 1], BF, name="ones_t")

            v_sb = [vpool.tile([128, GW], BF, name=f"v{c}") for c in range(NMT)]
            qraw = [qkpool.tile([128, C], BF, name=f"qr{h}") for h in range(HPC)]
            kraw = [qkpool.tile([128, C], BF, name=f"kr{h}") for h in range(HPC)]

            with tc.tile_pool(name="xtp", bufs=1) as xtp, \
                 tc.tile_pool(name="wqk", bufs=1) as wqk, \
                 tc.tile_pool(name="pap", bufs=8, space="PSUM") as pap:
                xt, wq_sb, wk_sb, wv_sb = [], [], [], []
                # k-interleaved so matmul k=0 can start after ~1MB of DMA
                for k in range(NKC):
                    ks = slice(128 * k, 128 * (k + 1))
                    tq = wqk.tile([128, GW], BF, name=f"wq{k}")
                    tk = wqk.tile([128, GW], BF, name=f"wk{k}")
                    tv = wqk.tile([128, GW], BF, name=f"wv{k}")
                    t = xtp.tile([128, C], BF, name=f"xt{k}")
                    nc.sync.dma_start(tq[:], wq_d[ks, :])
                    nc.sync.dma_start(tk[:], wk_d[ks, :])
                    nc.sync.dma_start(t[:], xt_d[ks, :])
                    nc.sync.dma_start(tv[:], wv_d[ks, :])
                    xt.append(t)
                    wq_sb.append(tq)
                    wk_sb.append(tk)
                    wv_sb.append(tv)
                    if k == 1:
                        # consts are not needed until RoPE; don't let them
                        # delay the first projection matmuls
                        nc.sync.dma_start(cs_t[:], cs_d[:])
                        nc.sync.dma_start(sn_t[:], sn_d[:])
                        nc.sync.dma_start(mskT_t[:], mskT_d[:])
                        nc.sync.dma_start(ones_t[:], ones_d[:])

                # ---- phase A: QK projections + in-place RoPE per head ----
                # k-outer order: first matmul only needs chunk 0 of the
                # weights + xT, so compute starts ~1MB into the DMA stream.
                # One ldweights serves 4 matmuls.  DVE/GpSimd are idle in
                # phase A, so the RoPE rotations are fully hidden.
                for h in range(HPC):
                    hs = slice(128 * h, 128 * (h + 1))
                    for dst, w_sb in ((qraw[h], wq_sb), (kraw[h], wk_sb)):
                        pq4 = [pap.tile([128, 512], F32, name=f"pq{n}",
                                        tag="pa") for n in range(4)]
                        for k in range(NKC):
                            for n in range(4):
                                nc.tensor.matmul(
                                    pq4[n][:], w_sb[k][:, hs],
                                    xt[k][:, 512 * n:512 * (n + 1)],
                                    start=(k == 0), stop=(k == NKC - 1))
                        for n in range(4):
                            ns = slice(512 * n, 512 * (n + 1))
                            nc.scalar.copy(dst[:, ns], pq4[n][:])
                        for n in range(4):
                            ns = slice(512 * n, 512 * (n + 1))
                            tmp = rtmp.tile([128, 512], BF, name="tmp",
                                            tag="rt")
                            nc.vector.tensor_copy(tmp[0:64, :],
                                                  dst[64:128, ns])
                            nc.vector.tensor_copy(tmp[64:128, :],
                                                  dst[0:64, ns])
                            m1 = rtmp.tile([128, 512], BF, name="m1", tag="rt")
                            nc.vector.tensor_mul(m1[:], dst[:, ns],
                                                 cs_t[:, ns])
                            m2 = rtmp.tile([128, 512], BF, name="m2", tag="rt")
                            nc.gpsimd.tensor_mul(m2[:], tmp[:], sn_t[:, ns])
                            nc.vector.tensor_add(dst[:, ns], m1[:], m2[:])
                    if h == 1:
                        # V projection in the middle so the phase-A pool
                        # release barrier lands right at the PE tail
                        for ct in range(NMT):
                            cts = slice(128 * ct, 128 * (ct + 1))
                            pv = pap.tile([128, GW], F32, name="pv", tag="pa")
                            for k in range(NKC):
                                nc.tensor.matmul(
                                    pv[:], xt[k][:, cts], wv_sb[k][:],
                                    start=(k == 0), stop=(k == NKC - 1))
                            nc.vector.tensor_copy(v_sb[ct][:], pv[:])


            # xt + w pools released here; attention pools reuse the space
            with tc.tile_pool(name="ptile", bufs=20) as ptp, \
                 tc.tile_pool(name="pmm", bufs=4, space="PSUM") as pmm, \
                 tc.tile_pool(name="attnT", bufs=1) as atp, \
                 tc.tile_pool(name="wop", bufs=1) as wop, \
                 tc.tile_pool(name="sums", bufs=4) as sump, \
                 tc.tile_pool(name="rbp", bufs=2) as rbp, \
                 tc.tile_pool(name="outsb", bufs=4) as outp, \
                 tc.tile_pool(name="rsps", bufs=2, space="PSUM") as rsps, \
                 tc.tile_pool(name="pvps", bufs=2, space="PSUM") as pvps:

                attnT = [atp.tile([128, C], BF, name=f"at{h}") for h in range(HPC)]
                wo_sb = []
                for hk in range(HPC):
                    t = wop.tile([128, D], BF, name=f"wo{hk}")
                    nc.sync.dma_start(t[:], wo_d[128 * hk:128 * (hk + 1), :])
                    wo_sb.append(t)

                # ---- attention, blocks outer so outproj interleaves ----
                qrot, krot = qraw, kraw  # rotated in place during phase A
                for I in range(NBLK):
                    qs = slice(512 * I, 512 * (I + 1))
                    nch = 4 * (I + 1)
                    for h in range(HPC):
                        hs = slice(128 * h, 128 * (h + 1))
                        pts = []
                        for c in range(nch):
                            ks = slice(128 * c, 128 * (c + 1))
                            psT = pmm.tile([128, 512], F32, name="psT",
                                           tag="pmm")
                            nc.tensor.matmul(psT[:], krot[h][:, ks],
                                             qrot[h][:, qs])
                            pt = ptp.tile([128, 512], BF, name="pt",
                                          tag="ptile")
                            nc.scalar.activation(
                                pt[:], psT[:],
                                mybir.ActivationFunctionType.Exp,
                                scale=float(SCALE))
                            j = c - 4 * I
                            if j >= 0:
                                nc.vector.tensor_mul(
                                    pt[:], pt[:],
                                    mskT_t[:, 512 * j:512 * (j + 1)])
                            pts.append(pt)
                        # row sums over the key axis via ones-matmul
                        rs = rsps.tile([1, 512], F32, name="rs", tag="rs")
                        for c in range(nch):
                            nc.tensor.matmul(rs[:], ones_t[:, 0:1], pts[c][:],
                                             start=(c == 0),
                                             stop=(c == nch - 1))
                        rec = sump.tile([1, 512], F32, name="rec", tag="sm")
                        nc.vector.tensor_copy(rec[:], rs[:])
                        rb = rbp.tile([128, 512], F32, name="rb", tag="rb")
                        nc.gpsimd.partition_broadcast(rb[:], rec[:])
                        nc.vector.reciprocal(rb[:], rb[:])
                        # PV
                        pvp = pvps.tile([128, 512], F32, name="pvp", tag="pv")
                        for c in range(nch):
                            nc.tensor.matmul(pvp[:], v_sb[c][:, hs], pts[c][:],
                                             start=(c == 0),
                                             stop=(c == nch - 1))
                        nc.vector.tensor_mul(attnT[h][:, qs], pvp[:], rb[:])

                    # ---- output projection, delayed one block so its
                    # attnT inputs are long-finished ----
                    for J in ([I - 1] if I < NBLK - 1 else [I - 1, I]):
                        if J < 0:
                            continue
                        for m in range(4 * J, 4 * (J + 1)):
                            ms = slice(128 * m, 128 * (m + 1))
                            for n in range(4):
                                ns = slice(512 * n, 512 * (n + 1))
                                po = pmm.tile([128, 512], F32, name="po",
                                              tag="pmm")
                                for hk in range(HPC):
                                    nc.tensor.matmul(po[:], attnT[hk][:, ms],
                                                     wo_sb[hk][:, ns],
                                                     start=(hk == 0),
                                                     stop=(hk == HPC - 1))
                                ot = outp.tile([128, 512], F32, name="ot",
                                               tag="ot")
                                nc.vector.tensor_copy(ot[:], po[:])
                                nc.sync.dma_start(out_d[ms, ns], ot[:])

    nc.compile()
    return nc


def _get_nc():
    if not _nc_cache:
        _nc_cache.append(_build_nc())
    return _nc_cache[0]


def _prep_inputs(x, freqs_cos, freqs_sin, Wq, Wk, Wv, Wo):
    # de-interleave permutation within each head's 128 output dims
    perm = np.concatenate([np.arange(0, HD, 2), np.arange(1, HD, 2)])

    cosT = np.ascontiguousarray(freqs_cos.T)  # [64, C]
    sinT = np.ascontiguousarray(freqs_sin.T)
    cs = np.concatenate([cosT, cosT], axis=0).astype(bf16)
    sn = np.concatenate([-sinT, sinT], axis=0).astype(bf16)

    # transposed causal masks for diagonal chunks: chunk c = 4I + j covers
    # keys 128c+p, queries 512I+cc; allowed iff cc >= 128j + p
    p = np.arange(128)[:, None]
    cc = np.arange(512)[None, :]
    mskT = np.concatenate(
        [(cc >= 128 * j + p) for j in range(4)], axis=1).astype(bf16)
    ones = np.ones((128, 1), dtype=bf16)

    xts = [np.ascontiguousarray(x[b].T).astype(bf16) for b in range(B)]

    in_maps = []
    for j in range(NCORE):
        b, g = divmod(j, HPC)
        rows = np.concatenate(
            [512 * g + 128 * hl + perm for hl in range(HPC)])
        rows_nop = np.arange(512 * g, 512 * (g + 1))
        in_maps.append({
            "xt": xts[b],
            "wq": np.ascontiguousarray(Wq[rows, :].T).astype(bf16),
            "wk": np.ascontiguousarray(Wk[rows, :].T).astype(bf16),
            "wv": np.ascontiguousarray(Wv[rows_nop, :].T).astype(bf16),
            "wo": np.ascontiguousarray(Wo[:, rows_nop].T).astype(bf16),
            "cs": cs,
            "sn": sn,
            "mskT": mskT,
            "ones": ones,
        })
    return in_maps


def kernel(x, freqs_cos, freqs_sin, Wq, Wk, Wv, Wo):
    x = np.asarray(x, dtype=np.float32)
    freqs_cos = np.asarray(freqs_cos, dtype=np.float32)
    freqs_sin = np.asarray(freqs_sin, dtype=np.float32)
    Wq = np.asarray(Wq, dtype=np.float32)
    Wk = np.asarray(Wk, dtype=np.float32)
    Wv = np.asarray(Wv, dtype=np.float32)
    Wo = np.asarray(Wo, dtype=np.float32)

    nc = _get_nc()
    in_maps = _prep_inputs(x, freqs_cos, freqs_sin, Wq, Wk, Wv, Wo)
    res = run_bass_kernel_spmd(nc, in_maps, list(range(NCORE)), trace=TRACE,
                               tmpdir=TMPDIR)
    LAST["res"] = res

    out = np.empty((B, C, D), dtype=np.float32)
    for b in range(B):
        acc = res.results[HPC * b]["out"].astype(np.float64)
        for g in range(1, HPC):
            acc += res.results[HPC * b + g]["out"]
        out[b] = acc.astype(np.float32)
    return out


# revision 16
# speedup vs baseline: 1.0308x; 1.0308x over previous
"""Causal self-attention with RoPE on 8 Trainium2 NeuronCores.

Sharding: tensor-parallel over heads (4 heads/core) x data-parallel over
batch (2 batches), 8 cores total.  Each core computes QKV projections for
its 4 heads from x[b].T, applies RoPE, runs causal attention, and produces
a partial output projection (row-parallel Wo); the host sums the 4 partials
per batch.

Per-core dataflow (all matmuls bf16 with fp32 PSUM accumulation):
  phase A: qT/kT = Wq_g @ xT (head-dim on partitions, real/imag
           de-interleaved via host-side weight row permutation so RoPE
           becomes partition-half operations), v = x @ Wv_g.T (natural)
  phase B: per head: RoPE rotate qT/kT; per 512-query block compute
           scores TRANSPOSED (k-major: lhsT=kT chunk, rhs=qT block) so
           softmax probabilities come out in the layout the PV matmul
           needs -- no PE transposes.  exp without max-subtraction
           (scores are bounded ~N(0,1)), causal mask on diagonal chunks,
           row-sums via a ones-vector matmul, normalization folded into
           the attnT copy-out as a column-broadcast multiply.
  phase C: partial out = attnT.T @ (Wo.T rows for this group) -> DRAM
"""

import sys

sys.path.insert(0, "/opt/trn_rl_repo")

import numpy as np
import ml_dtypes

import concourse.bass as bass
import concourse.mybir as mybir
import concourse.tile as tile
from concourse import bacc
from concourse.bass_utils import run_bass_kernel_spmd

B, C, D, H = 2, 2048, 2048, 16
HD = D // H            # 128 head dim
NCORE = 8
HPC = 4                # heads per core
GW = HPC * HD          # 512: per-core projection width
NKC = D // 128         # 16 contraction chunks
NMT = C // 128         # 16 query m-tiles
NBLK = C // 512        # 4 query blocks
SCALE = 1.0 / np.sqrt(HD)

bf16 = ml_dtypes.bfloat16
BF = mybir.dt.bfloat16
F32 = mybir.dt.float32

TRACE = False
TMPDIR = None
LAST = {}

_nc_cache = []


def _build_nc():
    nc = bacc.Bacc()

    xt_d = nc.declare_dram_parameter("xt", [D, C], BF, isOutput=False)
    wq_d = nc.declare_dram_parameter("wq", [D, GW], BF, isOutput=False)
    wk_d = nc.declare_dram_parameter("wk", [D, GW], BF, isOutput=False)
    wv_d = nc.declare_dram_parameter("wv", [D, GW], BF, isOutput=False)
    wo_d = nc.declare_dram_parameter("wo", [GW, D], BF, isOutput=False)
    cs_d = nc.declare_dram_parameter("cs", [128, C], BF, isOutput=False)
    sn_d = nc.declare_dram_parameter("sn", [128, C], BF, isOutput=False)
    mskT_d = nc.declare_dram_parameter("mskT", [128, 4 * 512], BF,
                                       isOutput=False)
    ones_d = nc.declare_dram_parameter("ones", [128, 1], BF, isOutput=False)
    out_d = nc.declare_dram_parameter("out", [C, D], F32, isOutput=True)

    with tile.TileContext(nc) as tc:
        with tc.tile_pool(name="consts", bufs=1) as cpool, \
             tc.tile_pool(name="vpool", bufs=1) as vpool, \
             tc.tile_pool(name="qkraw", bufs=1) as qkpool, \
             tc.tile_pool(name="rtmp", bufs=6) as rtmp:

            cs_t = cpool.tile([128, C], BF, name="cs_t")
            sn_t = cpool.tile([128, C], BF, name="sn_t")
            mskT_t = cpool.tile([128, 4 * 512], BF, name="mskT_t")
            ones_t = cpool.tile([128, 1], BF, name="ones_t")

            v_sb = [vpool.tile([128, GW], BF, name=f"v{c}") for c in range(NMT)]
            qraw = [qkpool.tile([128, C], BF, name=f"qr{h}") for h in range(HPC)]
            kraw = [qkpool.tile([128, C], BF, name=f"kr{h}") for h in range(HPC)]

            with tc.tile_pool(name="xtp", bufs=1) as xtp, \
                 tc.tile_pool(name="wqk", bufs=1) as wqk, \
                 tc.tile_pool(name="pap", bufs=8, space="PSUM") as pap:
                xt, wq_sb, wk_sb, wv_sb = [], [], [], []
                # k-interleaved so matmul k=0 can start after ~1MB of DMA
                for k in range(NKC):
                    ks = slice(128 * k, 128 * (k + 1))
                    tq = wqk.tile([128, GW], BF, name=f"wq{k}")
                    tk = wqk.tile([128, GW], BF, name=f"wk{k}")
                    tv = wqk.tile([128, GW], BF, name=f"wv{k}")
                    t = xtp.tile([128, C], BF, name=f"xt{k}")
                    nc.sync.dma_start(tq[:], wq_d[ks, :])
                    nc.sync.dma_start(tk[:], wk_d[ks, :])
                    nc.sync.dma_start(t[:], xt_d[ks, :])
                    nc.sync.dma_start(tv[:], wv_d[ks, :])
                    xt.append(t)
                    wq_sb.append(tq)
                    wk_sb.append(tk)
                    wv_sb.append(tv)
                    if k == 1:
                        # consts are not needed until RoPE; don't let them
                        # delay the first projection matmuls
                        nc.sync.dma_start(cs_t[:], cs_d[:])
                        nc.sync.dma_start(sn_t[:], sn_d[:])
                        nc.sync.dma_start(mskT_t[:], mskT_d[:])
                        nc.sync.dma_start(ones_t[:], ones_d[:])

                # ---- phase A: QK projections + in-place RoPE per head ----
                # k-outer order: first matmul only needs chunk 0 of the
                # weights + xT, so compute starts ~1MB into the DMA stream.
                # One ldweights serves 4 matmuls.  DVE/GpSimd are idle in
                # phase A, so the RoPE rotations are fully hidden.
                for h in range(HPC):
                    hs = slice(128 * h, 128 * (h + 1))
                    for dst, w_sb in ((qraw[h], wq_sb), (kraw[h], wk_sb)):
                        pq4 = [pap.tile([128, 512], F32, name=f"pq{n}",
                                        tag="pa") for n in range(4)]
                        for k in range(NKC):
                            for n in range(4):
                                nc.tensor.matmul(
                                    pq4[n][:], w_sb[k][:, hs],
                                    xt[k][:, 512 * n:512 * (n + 1)],
                                    start=(k == 0), stop=(k == NKC - 1))
                        for n in range(4):
                            ns = slice(512 * n, 512 * (n + 1))
                            nc.scalar.copy(dst[:, ns], pq4[n][:])
                        for n in range(4):
                            ns = slice(512 * n, 512 * (n + 1))
                            tmp = rtmp.tile([128, 512], BF, name="tmp",
                                            tag="rt")
                            nc.vector.tensor_copy(tmp[0:64, :],
                                                  dst[64:128, ns])
                            nc.vector.tensor_copy(tmp[64:128, :],
                                                  dst[0:64, ns])
                            m1 = rtmp.tile([128, 512], BF, name="m1", tag="rt")
                            nc.vector.tensor_mul(m1[:], dst[:, ns],
                                                 cs_t[:, ns])
                            m2 = rtmp.tile([128, 512], BF, name="m2", tag="rt")
                            nc.gpsimd.tensor_mul(m2[:], tmp[:], sn_t[:, ns])
                            nc.vector.tensor_add(dst[:, ns], m1[:], m2[:])
                    if h == HPC - 1:
                        # V projection last
                        for ct in range(NMT):
                            cts = slice(128 * ct, 128 * (ct + 1))
                            pv = pap.tile([128, GW], F32, name="pv", tag="pa")
                            for k in range(NKC):
                                nc.tensor.matmul(
                                    pv[:], xt[k][:, cts], wv_sb[k][:],
                                    start=(k == 0), stop=(k == NKC - 1))
                            nc.vector.tensor_copy(v_sb[ct][:], pv[:])


            # xt + w pools released here; attention pools reuse the space
            with tc.tile_pool(name="ptile", bufs=20) as ptp, \
                 tc.tile_pool(name="pmm", bufs=4, space="PSUM") as pmm, \
                 tc.tile_pool(name="attnT", bufs=1) as atp, \
                 tc.tile_pool(name="wop", bufs=1) as wop, \
                 tc.tile_pool(name="sums", bufs=4) as sump, \
                 tc.tile_pool(name="rbp", bufs=2) as rbp, \
                 tc.tile_pool(name="outsb", bufs=4) as outp, \
                 tc.tile_pool(name="rsps", bufs=2, space="PSUM") as rsps, \
                 tc.tile_pool(name="pvps", bufs=2, space="PSUM") as pvps:

                attnT = [atp.tile([128, C], BF, name=f"at{h}") for h in range(HPC)]
                wo_sb = []
                for hk in range(HPC):
                    t = wop.tile([128, D], BF, name=f"wo{hk}")
                    nc.sync.dma_start(t[:], wo_d[128 * hk:128 * (hk + 1), :])
                    wo_sb.append(t)

                # ---- attention, blocks outer so outproj interleaves ----
                qrot, krot = qraw, kraw  # rotated in place during phase A
                for I in range(NBLK):
                    qs = slice(512 * I, 512 * (I + 1))
                    nch = 4 * (I + 1)
                    for h in range(HPC):
                        hs = slice(128 * h, 128 * (h + 1))
                        pts = []
                        for c in range(nch):
                            ks = slice(128 * c, 128 * (c + 1))
                            psT = pmm.tile([128, 512], F32, name="psT",
                                           tag="pmm")
                            nc.tensor.matmul(psT[:], krot[h][:, ks],
                                             qrot[h][:, qs])
                            pt = ptp.tile([128, 512], BF, name="pt",
                                          tag="ptile")
                            nc.scalar.activation(
                                pt[:], psT[:],
                                mybir.ActivationFunctionType.Exp,
                                scale=float(SCALE))
                            j = c - 4 * I
                            if j >= 0:
                                nc.vector.tensor_mul(
                                    pt[:], pt[:],
                                    mskT_t[:, 512 * j:512 * (j + 1)])
                            pts.append(pt)
                        # row sums over the key axis via ones-matmul
                        rs = rsps.tile([1, 512], F32, name="rs", tag="rs")
                        for c in range(nch):
                            nc.tensor.matmul(rs[:], ones_t[:, 0:1], pts[c][:],
                                             start=(c == 0),
                                             stop=(c == nch - 1))
                        rec = sump.tile([1, 512], F32, name="rec", tag="sm")
                        nc.vector.tensor_copy(rec[:], rs[:])
                        rb = rbp.tile([128, 512], F32, name="rb", tag="rb")
                        nc.gpsimd.partition_broadcast(rb[:], rec[:])
                        nc.vector.reciprocal(rb[:], rb[:])
                        # PV
                        pvp = pvps.tile([128, 512], F32, name="pvp", tag="pv")
                        for c in range(nch):
                            nc.tensor.matmul(pvp[:], v_sb[c][:, hs], pts[c][:],
                                             start=(c == 0),
                                             stop=(c == nch - 1))
                        nc.vector.tensor_mul(attnT[h][:, qs], pvp[:], rb[:])

                    # ---- output projection, delayed one block so its
                    # attnT inputs are long-finished ----
                    for J in ([I - 1] if I < NBLK - 1 else [I - 1, I]):
                        if J < 0:
                            continue
                        for m in range(4 * J, 4 * (J + 1)):
                            ms = slice(128 * m, 128 * (m + 1))
                            for n in range(4):
                                ns = slice(512 * n, 512 * (n + 1))
                                po = pmm.tile([128, 512], F32, name="po",
                                              tag="pmm")
                                for hk in range(HPC):
                                    nc.tensor.matmul(po[:], attnT[hk][:, ms],
                                                     wo_sb[hk][:, ns],
                                                     start=(hk == 0),
                                                     stop=(hk == HPC - 1))
                                ot = outp.tile([128, 512], F32, name="ot",
                                               tag="ot")
                                nc.vector.tensor_copy(ot[:], po[:])
                                nc.sync.dma_start(out_d[ms, ns], ot[:])

    nc.compile()
    return nc


def _get_nc():
    if not _nc_cache:
        _nc_cache.append(_build_nc())
    return _nc_cache[0]


def _prep_inputs(x, freqs_cos, freqs_sin, Wq, Wk, Wv, Wo):
    # de-interleave permutation within each head's 128 output dims
    perm = np.concatenate([np.arange(0, HD, 2), np.arange(1, HD, 2)])

    cosT = np.ascontiguousarray(freqs_cos.T)  # [64, C]
    sinT = np.ascontiguousarray(freqs_sin.T)
    cs = np.concatenate([cosT, cosT], axis=0).astype(bf16)
    sn = np.concatenate([-sinT, sinT], axis=0).astype(bf16)

    # transposed causal masks for diagonal chunks: chunk c = 4I + j covers
    # keys 128c+p, queries 512I+cc; allowed iff cc >= 128j + p
    p = np.arange(128)[:, None]
    cc = np.arange(512)[None, :]
    mskT = np.concatenate(
        [(cc >= 128 * j + p) for j in range(4)], axis=1).astype(bf16)
    ones = np.ones((128, 1), dtype=bf16)

    xts = [np.ascontiguousarray(x[b].T).astype(bf16) for b in range(B)]

    in_maps = []
    for j in range(NCORE):
        b, g = divmod(j, HPC)
        rows = np.concatenate(
            [512 * g + 128 * hl + perm for hl in range(HPC)])
        rows_nop = np.arange(512 * g, 512 * (g + 1))
        in_maps.append({
            "xt": xts[b],
            "wq": np.ascontiguousarray(Wq[rows, :].T).astype(bf16),
            "wk": np.ascontiguousarray(Wk[rows, :].T).astype(bf16),
            "wv": np.ascontiguousarray(Wv[rows_nop, :].T).astype(bf16),
            "wo": np.ascontiguousarray(Wo[:, rows_nop].T).astype(bf16),
            "cs": cs,
            "sn": sn,
            "mskT": mskT,
            "ones": ones,
        })
    return in_maps


def kernel(x, freqs_cos, freqs_sin, Wq, Wk, Wv, Wo):
    x = np.asarray(x, dtype=np.float32)
    freqs_cos = np.asarray(freqs_cos, dtype=np.float32)
    freqs_sin = np.asarray(freqs_sin, dtype=np.float32)
    Wq = np.asarray(Wq, dtype=np.float32)
    Wk = np.asarray(Wk, dtype=np.float32)
    Wv = np.asarray(Wv, dtype=np.float32)
    Wo = np.asarray(Wo, dtype=np.float32)

    nc = _get_nc()
    in_maps = _prep_inputs(x, freqs_cos, freqs_sin, Wq, Wk, Wv, Wo)
    res = run_bass_kernel_spmd(nc, in_maps, list(range(NCORE)), trace=TRACE,
                               tmpdir=TMPDIR)
    LAST["res"] = res

    out = np.empty((B, C, D), dtype=np.float32)
    for b in range(B):
        acc = res.results[HPC * b]["out"].astype(np.float64)
        for g in range(1, HPC):
            acc += res.results[HPC * b + g]["out"]
        out[b] = acc.astype(np.float32)
    return out


# revision 18
# speedup vs baseline: 1.1088x; 1.0757x over previous
"""Causal self-attention with RoPE on 8 Trainium2 NeuronCores.

Sharding: tensor-parallel over heads (4 heads/core) x data-parallel over
batch (2 batches), 8 cores total.  Each core computes QKV projections for
its 4 heads from x[b].T, applies RoPE, runs causal attention, and produces
a partial output projection (row-parallel Wo); the host sums the 4 partials
per batch.

Per-core dataflow (all matmuls bf16 with fp32 PSUM accumulation):
  phase A: qT/kT = Wq_g @ xT (head-dim on partitions, real/imag
           de-interleaved via host-side weight row permutation so RoPE
           becomes partition-half operations), v = x @ Wv_g.T (natural)
  phase B: per head: RoPE rotate qT/kT; per 512-query block compute
           scores TRANSPOSED (k-major: lhsT=kT chunk, rhs=qT block) so
           softmax probabilities come out in the layout the PV matmul
           needs -- no PE transposes.  exp without max-subtraction
           (scores are bounded ~N(0,1)), causal mask on diagonal chunks,
           row-sums via a ones-vector matmul, normalization folded into
           the attnT copy-out as a column-broadcast multiply.
  phase C: partial out = attnT.T @ (Wo.T rows for this group) -> DRAM
"""

import sys

sys.path.insert(0, "/opt/trn_rl_repo")

import numpy as np
import ml_dtypes

import concourse.bass as bass
import concourse.mybir as mybir
import concourse.tile as tile
from concourse import bacc
from concourse.bass_utils import run_bass_kernel_spmd

B, C, D, H = 2, 2048, 2048, 16
HD = D // H            # 128 head dim
NCORE = 8
HPC = 4                # heads per core
GW = HPC * HD          # 512: per-core projection width
NKC = D // 128         # 16 contraction chunks
NMT = C // 128         # 16 query m-tiles
NBLK = C // 512        # 4 query blocks
SCALE = 1.0 / np.sqrt(HD)

bf16 = ml_dtypes.bfloat16
BF = mybir.dt.bfloat16
F32 = mybir.dt.float32

TRACE = False
TMPDIR = None
LAST = {}

_nc_cache = []


def _build_nc():
    nc = bacc.Bacc()

    xt_d = nc.declare_dram_parameter("xt", [D, C], BF, isOutput=False)
    wq_d = nc.declare_dram_parameter("wq", [D, GW], BF, isOutput=False)
    wk_d = nc.declare_dram_parameter("wk", [D, GW], BF, isOutput=False)
    wv_d = nc.declare_dram_parameter("wv", [D, GW], BF, isOutput=False)
    wo_d = nc.declare_dram_parameter("wo", [GW, D], BF, isOutput=False)
    cs_d = nc.declare_dram_parameter("cs", [128, C], BF, isOutput=False)
    sn_d = nc.declare_dram_parameter("sn", [128, C], BF, isOutput=False)
    mskT_d = nc.declare_dram_parameter("mskT", [128, 4 * 512], BF,
                                       isOutput=False)
    ones_d = nc.declare_dram_parameter("ones", [128, 1], BF, isOutput=False)
    out_d = nc.declare_dram_parameter("out", [C, D], F32, isOutput=True)

    with tile.TileContext(nc) as tc:
        with tc.tile_pool(name="consts", bufs=1) as cpool, \
             tc.tile_pool(name="vpool", bufs=1) as vpool, \
             tc.tile_pool(name="qkraw", bufs=1) as qkpool, \
             tc.tile_pool(name="rtmp", bufs=6) as rtmp:

            cs_t = cpool.tile([128, C], BF, name="cs_t")
            sn_t = cpool.tile([128, C], BF, name="sn_t")
            mskT_t = cpool.tile([128, 4 * 512], BF, name="mskT_t")
            ones_t = cpool.tile([128, 1], BF, name="ones_t")

            v_sb = [vpool.tile([128, GW], BF, name=f"v{c}") for c in range(NMT)]
            qraw = [qkpool.tile([128, C], BF, name=f"qr{h}") for h in range(HPC)]
            kraw = [qkpool.tile([128, C], BF, name=f"kr{h}") for h in range(HPC)]

            with tc.tile_pool(name="xtp", bufs=1) as xtp, \
                 tc.tile_pool(name="wqk", bufs=1) as wqk, \
                 tc.tile_pool(name="pap", bufs=8, space="PSUM") as pap:
                xt, wq_sb, wk_sb, wv_sb = [], [], [], []
                # k-interleaved so matmul k=0 can start after ~1MB of DMA
                for k in range(NKC):
                    ks = slice(128 * k, 128 * (k + 1))
                    tq = wqk.tile([128, GW], BF, name=f"wq{k}")
                    tk = wqk.tile([128, GW], BF, name=f"wk{k}")
                    tv = wqk.tile([128, GW], BF, name=f"wv{k}")
                    t = xtp.tile([128, C], BF, name=f"xt{k}")
                    nc.sync.dma_start(tq[:], wq_d[ks, :])
                    nc.sync.dma_start(tk[:], wk_d[ks, :])
                    nc.sync.dma_start(t[:], xt_d[ks, :])
                    nc.sync.dma_start(tv[:], wv_d[ks, :])
                    xt.append(t)
                    wq_sb.append(tq)
                    wk_sb.append(tk)
                    wv_sb.append(tv)
                    if k == 1:
                        # consts are not needed until RoPE; don't let them
                        # delay the first projection matmuls
                        nc.sync.dma_start(cs_t[:], cs_d[:])
                        nc.sync.dma_start(sn_t[:], sn_d[:])
                        nc.sync.dma_start(mskT_t[:], mskT_d[:])
                        nc.sync.dma_start(ones_t[:], ones_d[:])

                # ---- phase A: QK projections + in-place RoPE per head ----
                # k-outer order: first matmul only needs chunk 0 of the
                # weights + xT, so compute starts ~1MB into the DMA stream.
                # One ldweights serves 4 matmuls.  DVE/GpSimd are idle in
                # phase A, so the RoPE rotations are fully hidden.
                for h in range(HPC):
                    hs = slice(128 * h, 128 * (h + 1))
                    for dst, w_sb in ((qraw[h], wq_sb), (kraw[h], wk_sb)):
                        pq4 = [pap.tile([128, 512], F32, name=f"pq{n}",
                                        tag="pa") for n in range(4)]
                        for k in range(NKC):
                            for n in range(4):
                                nc.tensor.matmul(
                                    pq4[n][:], w_sb[k][:, hs],
                                    xt[k][:, 512 * n:512 * (n + 1)],
                                    start=(k == 0), stop=(k == NKC - 1))
                        for n in range(4):
                            ns = slice(512 * n, 512 * (n + 1))
                            nc.scalar.copy(dst[:, ns], pq4[n][:])
                        for n in range(4):
                            ns = slice(512 * n, 512 * (n + 1))
                            tmp = rtmp.tile([128, 512], BF, name="tmp",
                                            tag="rt")
                            nc.vector.tensor_copy(tmp[0:64, :],
                                                  dst[64:128, ns])
                            nc.vector.tensor_copy(tmp[64:128, :],
                                                  dst[0:64, ns])
                            m1 = rtmp.tile([128, 512], BF, name="m1", tag="rt")
                            nc.vector.tensor_mul(m1[:], dst[:, ns],
                                                 cs_t[:, ns])
                            m2 = rtmp.tile([128, 512], BF, name="m2", tag="rt")
                            nc.gpsimd.tensor_mul(m2[:], tmp[:], sn_t[:, ns])
                            nc.vector.tensor_add(dst[:, ns], m1[:], m2[:])
                    if h == HPC - 1:
                        # V projection last
                        for ct in range(NMT):
                            cts = slice(128 * ct, 128 * (ct + 1))
                            pv = pap.tile([128, GW], F32, name="pv", tag="pa")
                            for k in range(NKC):
                                nc.tensor.matmul(
                                    pv[:], xt[k][:, cts], wv_sb[k][:],
                                    start=(k == 0), stop=(k == NKC - 1))
                            nc.vector.tensor_copy(v_sb[ct][:], pv[:])


            # xt + w pools released here; attention pools reuse the space
            with tc.tile_pool(name="ptile", bufs=20) as ptp, \
                 tc.tile_pool(name="pmm", bufs=4, space="PSUM") as pmm, \
                 tc.tile_pool(name="attnT", bufs=1) as atp, \
                 tc.tile_pool(name="wop", bufs=1) as wop, \
                 tc.tile_pool(name="sums", bufs=4) as sump, \
                 tc.tile_pool(name="rbp", bufs=2) as rbp, \
                 tc.tile_pool(name="outsb", bufs=4) as outp, \
                 tc.tile_pool(name="rsps", bufs=1, space="PSUM") as rsps, \
                 tc.tile_pool(name="pvps", bufs=3, space="PSUM") as pvps:

                attnT = [atp.tile([128, C], BF, name=f"at{h}") for h in range(HPC)]
                wo_sb = []
                for hk in range(HPC):
                    t = wop.tile([128, D], BF, name=f"wo{hk}")
                    nc.sync.dma_start(t[:], wo_d[128 * hk:128 * (hk + 1), :])
                    wo_sb.append(t)

                # ---- attention, blocks outer so outproj interleaves ----
                qrot, krot = qraw, kraw  # rotated in place during phase A
                for I in range(NBLK):
                    qs = slice(512 * I, 512 * (I + 1))
                    nch = 4 * (I + 1)
                    for h in range(HPC):
                        hs = slice(128 * h, 128 * (h + 1))
                        pts = []
                        for c in range(nch):
                            ks = slice(128 * c, 128 * (c + 1))
                            psT = pmm.tile([128, 512], F32, name="psT",
                                           tag="pmm")
                            nc.tensor.matmul(psT[:], krot[h][:, ks],
                                             qrot[h][:, qs])
                            pt = ptp.tile([128, 512], BF, name="pt",
                                          tag="ptile")
                            nc.scalar.activation(
                                pt[:], psT[:],
                                mybir.ActivationFunctionType.Exp,
                                scale=float(SCALE))
                            j = c - 4 * I
                            if j >= 0:
                                nc.vector.tensor_mul(
                                    pt[:], pt[:],
                                    mskT_t[:, 512 * j:512 * (j + 1)])
                            pts.append(pt)
                        # row sums over the key axis via ones-matmul
                        rs = rsps.tile([1, 512], F32, name="rs", tag="rs")
                        for c in range(nch):
                            nc.tensor.matmul(rs[:], ones_t[:, 0:1], pts[c][:],
                                             start=(c == 0),
                                             stop=(c == nch - 1))
                        rec = sump.tile([1, 512], F32, name="rec", tag="sm")
                        nc.vector.tensor_copy(rec[:], rs[:])
                        rb = rbp.tile([128, 512], F32, name="rb", tag="rb")
                        nc.gpsimd.partition_broadcast(rb[:], rec[:])
                        nc.vector.reciprocal_approx_fast(out=rb[:], in_=rb[:])
                        # PV
                        pvp = pvps.tile([128, 512], F32, name="pvp", tag="pv")
                        for c in range(nch):
                            nc.tensor.matmul(pvp[:], v_sb[c][:, hs], pts[c][:],
                                             start=(c == 0),
                                             stop=(c == nch - 1))
                        nc.vector.tensor_mul(attnT[h][:, qs], pvp[:], rb[:])

                    # ---- output projection, delayed one block so its
                    # attnT inputs are long-finished ----
                    for J in ([I - 1] if I < NBLK - 1 else [I - 1, I]):
                        if J < 0:
                            continue
                        for m in range(4 * J, 4 * (J + 1)):
                            ms = slice(128 * m, 128 * (m + 1))
                            for n in range(4):
                                ns = slice(512 * n, 512 * (n + 1))
                                po = pmm.tile([128, 512], F32, name="po",
                                              tag="pmm")
                                for hk in range(HPC):
                                    nc.tensor.matmul(po[:], attnT[hk][:, ms],
                                                     wo_sb[hk][:, ns],
                                                     start=(hk == 0),
                                                     stop=(hk == HPC - 1))
                                ot = outp.tile([128, 512], F32, name="ot",
                                               tag="ot")
                                nc.vector.tensor_copy(ot[:], po[:])
                                nc.sync.dma_start(out_d[ms, ns], ot[:])

    nc.compile()
    return nc


def _get_nc():
    if not _nc_cache:
        _nc_cache.append(_build_nc())
    return _nc_cache[0]


def _prep_inputs(x, freqs_cos, freqs_sin, Wq, Wk, Wv, Wo):
    # de-interleave permutation within each head's 128 output dims
    perm = np.concatenate([np.arange(0, HD, 2), np.arange(1, HD, 2)])

    cosT = np.ascontiguousarray(freqs_cos.T)  # [64, C]
    sinT = np.ascontiguousarray(freqs_sin.T)
    cs = np.concatenate([cosT, cosT], axis=0).astype(bf16)
    sn = np.concatenate([-sinT, sinT], axis=0).astype(bf16)

    # transposed causal masks for diagonal chunks: chunk c = 4I + j covers
    # keys 128c+p, queries 512I+cc; allowed iff cc >= 128j + p
    p = np.arange(128)[:, None]
    cc = np.arange(512)[None, :]
    mskT = np.concatenate(
        [(cc >= 128 * j + p) for j in range(4)], axis=1).astype(bf16)
    ones = np.ones((128, 1), dtype=bf16)

    xts = [np.ascontiguousarray(x[b].T).astype(bf16) for b in range(B)]

    in_maps = []
    for j in range(NCORE):
        b, g = divmod(j, HPC)
        rows = np.concatenate(
            [512 * g + 128 * hl + perm for hl in range(HPC)])
        rows_nop = np.arange(512 * g, 512 * (g + 1))
        in_maps.append({
            "xt": xts[b],
            "wq": np.ascontiguousarray(Wq[rows, :].T).astype(bf16),
            "wk": np.ascontiguousarray(Wk[rows, :].T).astype(bf16),
            "wv": np.ascontiguousarray(Wv[rows_nop, :].T).astype(bf16),
            "wo": np.ascontiguousarray(Wo[:, rows_nop].T).astype(bf16),
            "cs": cs,
            "sn": sn,
            "mskT": mskT,
            "ones": ones,
        })
    return in_maps


def kernel(x, freqs_cos, freqs_sin, Wq, Wk, Wv, Wo):
    x = np.asarray(x, dtype=np.float32)
    freqs_cos = np.asarray(freqs_cos, dtype=np.float32)
    freqs_sin = np.asarray(freqs_sin, dtype=np.float32)
    Wq = np.asarray(Wq, dtype=np.float32)
    Wk = np.asarray(Wk, dtype=np.float32)
    Wv = np.asarray(Wv, dtype=np.float32)
    Wo = np.asarray(Wo, dtype=np.float32)

    nc = _get_nc()
    in_maps = _prep_inputs(x, freqs_cos, freqs_sin, Wq, Wk, Wv, Wo)
    res = run_bass_kernel_spmd(nc, in_maps, list(range(NCORE)), trace=TRACE,
                               tmpdir=TMPDIR)
    LAST["res"] = res

    out = np.empty((B, C, D), dtype=np.float32)
    for b in range(B):
        acc = res.results[HPC * b]["out"].astype(np.float64)
        for g in range(1, HPC):
            acc += res.results[HPC * b + g]["out"]
        out[b] = acc.astype(np.float32)
    return out
